# revision 15
# baseline (speedup 1.0000x reference)
"""Trainium2 Bass kernel for nn_CrossPatchContextModule.

Math (per batch b):
    hi = x @ W1[:D];  hj = x @ W1[D:]
    scores[i,j] = W2 . relu(hi[i] + hj[j] + b1) + b2     (diag forced to 0)
    w = softmax(scores, axis=j)
    out = x + LN(w @ x @ Wp + bp) * gamma + beta

Sharding: data-parallel over batch. B=8 batches -> 8 NeuronCores, one
batch per core, all parameters replicated. No collectives.

Per-core algorithm (N=D=256, P=128 partitions):
  * hjW[e,j] = (W1[D:]^T x^T)[e,j] (fp16, streamed operand) and
    hibW[e,i] = (W1[:D]^T x^T)[e,i] + b1[e] (fp32, per-partition scalar
    source), each as two 128-row e-chunks.
  * Pairwise tiles are emitted per i (transposed relative to the score
    matrix): T_i[e, (c,j)] = relu(hjW[c][e,j] + hibW[c][e,i]) in ONE
    fused tensor_scalar (DVE / Pool) or activation-Relu (ACT) op per
    (i, c), [128, 256] fp16. i's are distributed over the three
    elementwise-capable engines in proportion to their per-op rates
    (DVE ~127ns, ACT ~398ns, Pool ~451ns per [128,256] op), two i's
    packed per SBUF tile to halve pool-recycle semaphore traffic.
  * scores^T[:, i] = sum_e W2_e T_i[e, :]: the PE contracts with T_i as
    the STATIONARY operand (lhsT [128e, 128j]) and the W2 e-chunk
    column [128, 1] as the moving operand, accumulating the two
    e-chunks into one PSUM column psum_sT[:, g*N+i] (j = p + 128g).
    The moving free size is 1, so the contraction occupies the PE for
    only ~1 cycle per matmul; weight loads ride the PE's spare
    capacity. Each column's accumulation opens with a one-hot matmul
    that plants -30 on the diagonal entry (j == i), so exp underflows
    fp16 to an exact 0 there and no masking ever touches the vector
    engines. PSUM holds scores TRANSPOSED: [j(part), i(free)].
  * softmax: b2 shifts every in-row score equally so it cancels in
    softmax EXCEPT at the zeroed diagonal. Exp runs straight on the
    raw PSUM scores; the diagonal's exp(-b2) re-enters as constant PE
    accumulations: a memset (exp(-b2)/128) tile into the row-sum S and
    exp(-b2)*I @ x^T into ctx. Row sums come from a ones-column PE
    matvec over ew^T; normalization by 1/S_i is folded into the proj
    output as (ctx_raw @ Wp) * r_i + bp in one scalar_tensor_tensor.
  * The i<128 half of the whole softmax/ctx/proj/LayerNorm/output
    pipeline is emitted MID-LOOP: its score columns complete halfway
    through the pairwise stage, so that half's tail (including its
    output DMA) hides entirely under the elementwise phase. Only the
    i>=128 half runs after the last pairwise tile.
  * ctx^T[d,i] = x^T-chunks (lhsT) @ ew^T, proj[i,e] = ctx^T (lhsT) @
    Wp; LayerNorm via bn_stats/bn_aggr, rstd = Exp(-0.5*Ln(var+eps)),
    y = (pb - mu)*rstd in one two-scalar tensor_scalar, residual add,
    fp16 output DMA (host widens to fp32).

T tiles, ew, the matmul operands and the output are fp16 (DVE
tensor_scalar gets the 4x 16-bit perf mode; the PE streams fp16 at
1 col/cycle). Scores accumulate in fp32 PSUM; LayerNorm statistics and
the residual math stay fp32.
"""

import math

import numpy as np
from contextlib import ExitStack

import concourse.bass as bass
import concourse.bacc as bacc
import concourse.tile as tile
from concourse import mybir
from concourse.bass_utils import run_bass_kernel_spmd

B, N, D = 8, 256, 256
P = 128
LN_EPS = 1e-5
F32 = mybir.dt.float32
F16 = mybir.dt.float16
AF = mybir.ActivationFunctionType
OP = mybir.AluOpType

# Tile schedule for the pairwise stage: tiles hold 1-2 i's, all ops of a
# tile on one engine. Engine shares match per-[128,256]-op rates (DVE
# 127ns, ACT 398ns, Pool 451ns): 160 i's DVE, 51 ACT, 45 Pool. Tiles are
# interleaved by earliest-virtual-finish so the three engine streams and
# the PE's in-order column consumption pace together.
def _tile_sched():
    per_op = {"v": 0.127, "a": 0.398, "p": 0.4505}
    tiles = (
        [("v", 2)] * 79 + [("a", 2)] * 25 + [("a", 1)] + [("p", 2)] * 23 + [("p", 1)]
    )
    by_eng = {k: [t for t in tiles if t[0] == k] for k in per_op}
    t_eng = {k: 0.0 for k in per_op}
    idx = {k: 0 for k in per_op}
    order = []
    next_i = 0
    for _ in range(len(tiles)):
        # pick engine whose next tile would finish earliest
        k = min(
            (k for k in per_op if idx[k] < len(by_eng[k])),
            key=lambda k: t_eng[k] + 2 * by_eng[k][idx[k]][1] * per_op[k],
        )
        n_i = by_eng[k][idx[k]][1]
        idx[k] += 1
        t_eng[k] += 2 * n_i * per_op[k]
        order.append((k, list(range(next_i, next_i + n_i))))
        next_i += n_i
    assert next_i == N
    return order


def _single_act_table(arch):
    """All activation funcs this kernel uses (Relu/Identity/Copy/Exp/Ln)
    live in set 6 (natural_log_exp_and_others). The stock greedy table
    placer picks sets 0/5 and ping-pongs 5 table loads (~2.7us each on
    HW); masking every other set forces one load of set 6. Canonical set
    indices are preserved (walrus maps id -> act.json by position)."""
    import concourse.hw_specs as hw_specs

    tabs = hw_specs.get_activation_tables(arch)
    keep = "natural_log_exp_and_others"
    need = {AF.Relu, AF.Identity, AF.Copy, AF.Exp, AF.Ln}
    if keep not in tabs or not need.issubset(tabs[keep]):
        return tabs  # fall back to the stock placement
    return {name: (funcs if name == keep else set()) for name, funcs in tabs.items()}


def _build_program(b2_val: float, use_gamma: bool, use_beta: bool):
    nc = bacc.Bacc("TRN2", target_bir_lowering=False, debug=False)

    xb_d = nc.dram_tensor("xb", [N, D], F32, kind="ExternalInput")
    xt16_d = nc.dram_tensor("xt16", [P, 2 * N], F16, kind="ExternalInput")
    x16_d = nc.dram_tensor("x16", [N, D], F16, kind="ExternalInput")
    w1a_d = nc.dram_tensor("w1a", [P, 2 * D], F16, kind="ExternalInput")
    w1b_d = nc.dram_tensor("w1b", [P, 2 * D], F16, kind="ExternalInput")
    b1c_d = nc.dram_tensor("b1c", [P, 2], F32, kind="ExternalInput")
    w2c_d = nc.dram_tensor("w2c", [P, 2], F16, kind="ExternalInput")
    # negid = (-30 * exp(b2)) * I, cid = exp(-b2) * I: negid[:, m] x cid
    # one-hot columns plant -30 on score diagonals; cid @ x^T re-adds the
    # diagonal softmax weight exp(-b2) into ctx.
    negid_d = nc.dram_tensor("negid", [P, P], F16, kind="ExternalInput")
    cid_d = nc.dram_tensor("cid", [P, P], F16, kind="ExternalInput")
    wp_d = nc.dram_tensor("wp", [D, D], F16, kind="ExternalInput")
    bpr_d = nc.dram_tensor("bpr", [P, D], F32, kind="ExternalInput")
    xpb_d = (
        nc.dram_tensor("xpb", [N, D], F32, kind="ExternalInput")
        if use_beta
        else None
    )
    gam_d = (
        nc.dram_tensor("gamr", [P, D], F32, kind="ExternalInput")
        if use_gamma
        else None
    )
    out_d = nc.dram_tensor("out", [N, D], F16, kind="ExternalOutput")

    with tile.TileContext(nc) as tc, ExitStack() as ctx:
        const = ctx.enter_context(tc.tile_pool(name="const", bufs=1))
        tpool = {
            "v": ctx.enter_context(tc.tile_pool(name="tv", bufs=12)),
            "a": ctx.enter_context(tc.tile_pool(name="ta", bufs=6)),
            "p": ctx.enter_context(tc.tile_pool(name="tp", bufs=6)),
        }
        ppre = ctx.enter_context(tc.tile_pool(name="ppre", bufs=2, space="PSUM"))
        pctx = ctx.enter_context(tc.tile_pool(name="pctx", bufs=1, space="PSUM"))
        pproj = ctx.enter_context(tc.tile_pool(name="pproj", bufs=1, space="PSUM"))
        pS = ctx.enter_context(tc.tile_pool(name="pS", bufs=1, space="PSUM"))
        pscore = ctx.enter_context(
            tc.tile_pool(name="pscore", bufs=1, space="PSUM")
        )

        # per-partition scalar constants for activation bias operands
        zero1 = const.tile([P, 1], F32)
        nc.vector.memset(zero1, 0.0)
        eps1 = const.tile([P, 1], F32)
        nc.vector.memset(eps1, LN_EPS)
        ones16 = const.tile([P, 1], F16)
        nc.vector.memset(ones16, 1.0)
        # cfill @ ones adds the diagonal's exp(-b2) into the softmax sums
        cfill = const.tile([P, P], F16)
        nc.vector.memset(cfill, math.exp(-b2_val) / P)
        # dummy activation with no data deps: forces the one ACT table load
        # (natural_log_exp set, ~1.3us) to run at t~0 instead of gating the
        # first real ACT op
        warm = const.tile([P, 1], F32)
        nc.scalar.activation(warm, zero1, AF.Relu, bias=zero1[:, 0:1])

        # ------- input DMAs needed before the main loop -------------------
        # x^T and the fp16 copy of x are host-side layout transforms of the
        # per-core shard (no FLOPs) - DMA them directly.
        # chunk pairs merged into single DMAs (DMA init latency dominates
        # these small transfers): tile [128, 2, 256], block c = rows
        # [128c, 128c+128) of the dram tensor
        # packed [P, 2X] layouts: one contiguous 1024B run per partition
        # per DMA descriptor (512B runs pay a 2x latency multiplier)
        xT_all = const.tile([P, 2 * N], F16)
        nc.sync.dma_start(xT_all, xt16_d[:])
        xT = [xT_all[:, c * N : (c + 1) * N] for c in range(2)]
        w1a_all = const.tile([P, 2 * D], F16)
        nc.scalar.dma_start(w1a_all, w1a_d[:])
        w1a = [w1a_all[:, c * D : (c + 1) * D] for c in range(2)]
        w1b_all = const.tile([P, 2 * D], F16)
        nc.gpsimd.dma_start(w1b_all, w1b_d[:])
        w1b = [w1b_all[:, c * D : (c + 1) * D] for c in range(2)]
        b1c = const.tile([P, 2], F32)
        nc.gpsimd.dma_start(b1c, b1c_d[:])
        # w2c/negid/cid only feed the PE score matmuls, which trail the
        # elementwise engines anyway - they can arrive late.
        w2c = const.tile([P, 2], F16)
        nc.gpsimd.dma_start(w2c, w2c_d[:])
        negid = const.tile([P, P], F16)
        nc.sync.dma_start(negid, negid_d[:])
        cid = const.tile([P, P], F16)
        nc.sync.dma_start(cid, cid_d[:])
        # ---------------- hjW (fp16 stream), hibW (fp32 scalars) ----------
        # hjW[e,j] = sum_d W1b[d,e] x[j,d] ; hibW[e,i] = sum_d W1a[d,e]
        # x[i,d] + b1[e]. ec=0 chunks first so the c=0 pairwise ops can
        # start while the ec=1 matmuls still run.
        hjW = [const.tile([P, N], F16, tag=f"hjW{c}", name=f"hjW{c}") for c in range(2)]
        hibW = [const.tile([P, N], F32, tag=f"hibW{c}", name=f"hibW{c}") for c in range(2)]
        for ec in range(2):
            ph = ppre.tile([P, N], F32, tag="mm")
            for dc in range(2):
                nc.tensor.matmul(
                    ph,
                    w1b[dc][:, ec * P : (ec + 1) * P],
                    xT[dc],
                    start=(dc == 0),
                    stop=(dc == 1),
                )
            nc.scalar.activation(hjW[ec], ph, AF.Identity, bias=zero1[:, 0:1])
            ph2 = ppre.tile([P, N], F32, tag="mm")
            for dc in range(2):
                nc.tensor.matmul(
                    ph2,
                    w1a[dc][:, ec * P : (ec + 1) * P],
                    xT[dc],
                    start=(dc == 0),
                    stop=(dc == 1),
                )
            nc.vector.tensor_scalar(
                out=hibW[ec], in0=ph2, scalar1=b1c[:, ec : ec + 1],
                scalar2=None, op0=OP.add,
            )

        # ---------------- pairwise scores (transposed) --------------------
        # psum_sT[p, g*256+i] = scores[i, j=p+128g] (diag = -30)
        # tile layout per i: [c0: j 0..256 | c1: j 0..256], each 256 wide
        psum_sT = pscore.tile([P, 2 * N], F32)
        engs = {"v": nc.vector, "a": nc.scalar, "p": nc.gpsimd}

        def emit_op(ek, tt, k, i, c):
            dst = tt[:, k, c, :]
            if ek == "a":
                nc.scalar.activation(
                    dst, hjW[c], AF.Relu, bias=hibW[c][:, i : i + 1]
                )
            else:
                engs[ek].tensor_scalar(
                    out=dst,
                    in0=hjW[c],
                    scalar1=hibW[c][:, i : i + 1],
                    scalar2=0.0,
                    op0=OP.add,
                    op1=OP.max,
                )

        def emit_mms(tt, k, i):
            gd = i // P
            for g in range(2):
                col = psum_sT[:, g * N + i : g * N + i + 1]
                if g == gd:
                    nc.tensor.matmul(
                        col, cid, negid[:, i % P : i % P + 1],
                        start=True, stop=False, skip_group_check=True,
                    )
                for c in range(2):
                    nc.tensor.matmul(
                        col,
                        tt[:, k, c, g * P : g * P + P],
                        w2c[:, c : c + 1],
                        start=(c == 0 and g != gd),
                        stop=(c == 1),
                        skip_group_check=True,
                    )

        # ---- tail stages, per i-half (ic=0 emitted mid-loop) -------------
        ew = const.tile([P, 2, N], F16)
        S_ps = pS.tile([P, 2], F32)
        recip = const.tile([P, 2], F32)
        ctxT = [const.tile([P, N], F16, tag=f"ctxT{c}", name=f"ctxT{c}") for c in range(2)]
        pcs = [None, None]
        tail_state = {}

        def emit_tail(ic):
            isl = slice(ic * P, (ic + 1) * P)
            ps2 = psum_sT[:].rearrange("p (g n) -> p g n", g=2)
            nc.scalar.activation(
                ew[:, :, isl], ps2[:, :, isl], AF.Exp, bias=zero1[:, 0:1]
            )
            # softmax denominators: S[i] = sum_j ew^T[j, i] + exp(-b2)
            for g in range(2):
                nc.tensor.matmul(
                    S_ps[:, ic : ic + 1],
                    ew[:, g, isl],
                    ones16[:, 0:1],
                    start=(g == 0),
                    stop=False,
                    skip_group_check=True,
                )
            nc.tensor.matmul(
                S_ps[:, ic : ic + 1], cfill, ones16[:, 0:1],
                start=False, stop=True, skip_group_check=True,
            )
            nc.vector.reciprocal(recip[:, ic : ic + 1], S_ps[:, ic : ic + 1])
            # ctx^T[d, i] = sum_j x[j, d] ew^T[j, i] + exp(-b2) x^T[d, i]
            for dc in range(2):
                if pcs[dc] is None:
                    pcs[dc] = pctx.tile([P, N], F32, tag=f"pc{dc}", name=f"pc{dc}")
                pc = pcs[dc]
                for g in range(2):
                    nc.tensor.matmul(
                        pc[:, isl],
                        x16[g][:, dc * P : (dc + 1) * P],
                        ew[:, g, isl],
                        start=(g == 0),
                        stop=False,
                        skip_group_check=True,
                    )
                nc.tensor.matmul(
                    pc[:, isl], cid, xT[dc][:, isl],
                    start=False, stop=True, skip_group_check=True,
                )
                # both copies on DVE: ACT's queue holds the Exp that
                # precedes them, so DVE finishes the pair sooner
                nc.vector.tensor_copy(ctxT[dc][:, isl], pc[:, isl])
            # proj (raw), then pb = proj*r_i + bp, LayerNorm, residual
            pp = pproj.tile([P, N], F32, tag="pp", name=f"pp{ic}")
            for dc in range(2):
                nc.tensor.matmul(
                    pp,
                    ctxT[dc][:, isl],
                    wp16[dc],
                    start=(dc == 0),
                    stop=(dc == 1),
                )
            pb = const.tile([P, D], F32, tag=f"pb{ic}", name=f"pb{ic}")
            nc.vector.scalar_tensor_tensor(
                out=pb, in0=pp, scalar=recip[:, ic : ic + 1], in1=bpr,
                op0=OP.mult, op1=OP.add,
            )
            st = const.tile([P, 6], F32, tag=f"st{ic}", name=f"st{ic}")
            nc.vector.bn_stats(st, pb)
            mv = const.tile([P, 2], F32, tag=f"mv{ic}", name=f"mv{ic}")
            nc.vector.bn_aggr(mv, st)
            lnv = const.tile([P, 1], F32, tag=f"lnv{ic}", name=f"lnv{ic}")
            nc.scalar.activation(lnv, mv[:, 1:2], AF.Ln, bias=eps1[:, 0:1])
            rstd = const.tile([P, 1], F32, tag=f"rstd{ic}", name=f"rstd{ic}")
            nc.scalar.activation(rstd, lnv, AF.Exp, bias=zero1[:, 0:1], scale=-0.5)
            # y = (pb - mu) * rstd in one two-scalar op
            tt2 = const.tile([P, D], F32, tag=f"tt{ic}", name=f"tt{ic}")
            tt_eng = nc.vector
            tt_eng.tensor_scalar(
                out=tt2,
                in0=pb,
                scalar1=mv[:, 0:1],
                scalar2=rstd[:, 0:1],
                op0=OP.subtract,
                op1=OP.mult,
            )
            if use_gamma:
                tg = const.tile([P, D], F32, tag=f"tg{ic}", name=f"tg{ic}")
                nc.vector.tensor_tensor(out=tg, in0=tt2, in1=gam, op=OP.mult)
                tt2 = tg
            ot = const.tile([P, D], F16, tag=f"ot{ic}", name=f"ot{ic}")
            ot_eng = nc.vector
            ot_eng.tensor_tensor(out=ot, in0=tt2, in1=xpb[ic], op=OP.add)
            nc.sync.dma_start(out_d[ic * P : (ic + 1) * P, :], ot)

        sched = _tile_sched()
        # find the tile index after which all i<128 columns are complete,
        # plus a safety margin so the PE has surely caught up
        done_i = 0
        tail0_at = None
        for m, (ek, ii) in enumerate(sched):
            done_i = max(done_i, max(ii) + 1)
            if done_i >= P and tail0_at is None:
                tail0_at = m + 4
        # stagger the first DVE tiles: their c=0 ops run while the ec=1
        # projections still compute, so DVE never stalls on hjW[1]
        pending = []  # staggered DVE tiles whose c=1 ops are deferred
        n_stag = 0
        tail_dmas_emitted = False
        for m, (ek, ii) in enumerate(sched):
            if m == 2 and not tail_dmas_emitted:
                # DMAs needed by the tail; emitted once the front DMAs of
                # each queue are already in flight so these queue behind
                tail_dmas_emitted = True
                x = [
                    const.tile([P, D], F32, tag=f"x{c}", name=f"x{c}")
                    for c in range(2)
                ]
                nc.sync.dma_start(x[0], xb_d[0:P, :])
                nc.sync.dma_start(x[1], xb_d[P : 2 * P, :])
                x16_all = const.tile([P, 2, D], F16)
                nc.sync.dma_start(
                    x16_all, x16_d[:].rearrange("(c p) n -> p c n", p=P)
                )
                x16 = [x16_all[:, c, :] for c in range(2)]
                wp16_all = const.tile([P, 2, D], F16)
                nc.sync.dma_start(
                    wp16_all, wp_d[:].rearrange("(c p) n -> p c n", p=P)
                )
                wp16 = [wp16_all[:, c, :] for c in range(2)]
                bpr = const.tile([P, D], F32)
                nc.sync.dma_start(bpr, bpr_d[:])
                if use_beta:
                    xpb = [
                        const.tile([P, D], F32, tag=f"xpb{c}", name=f"xpb{c}")
                        for c in range(2)
                    ]
                    for c in range(2):
                        nc.sync.dma_start(xpb[c], xpb_d[c * P : (c + 1) * P, :])
                else:
                    xpb = x
                if use_gamma:
                    gam = const.tile([P, D], F32)
                    nc.sync.dma_start(gam, gam_d[:])
            tt = tpool[ek].tile([P, len(ii), 2, N], F16, tag=f"T{ek}{len(ii)}")
            if ek == "v" and n_stag < 8:
                for k, i in enumerate(ii):
                    emit_op(ek, tt, k, i, 0)
                pending.append((ek, tt, list(enumerate(ii))))
                n_stag += 1
                continue
            for k, i in enumerate(ii):
                emit_op(ek, tt, k, i, 0)
                emit_op(ek, tt, k, i, 1)
            for k, i in enumerate(ii):
                emit_mms(tt, k, i)
            if pending and n_stag == 8:
                for pek, ptt, pki in pending:
                    for k, i in pki:
                        emit_op(pek, ptt, k, i, 1)
                    for k, i in pki:
                        emit_mms(ptt, k, i)
                pending = []
                n_stag = 9
            if m == tail0_at:
                emit_tail(0)
        emit_tail(1)

    import concourse.bacc as _bacc_mod

    orig = _bacc_mod.get_activation_tables
    _bacc_mod.get_activation_tables = _single_act_table
    try:
        nc.compile()
    finally:
        _bacc_mod.get_activation_tables = orig
    return nc


_cache = {}


def _get_program(b2_val: float, use_gamma: bool, use_beta: bool):
    key = (b2_val, use_gamma, use_beta)
    if key not in _cache:
        _cache[key] = _build_program(b2_val, use_gamma, use_beta)
    return _cache[key]


def _host_inputs(inputs):
    x = np.ascontiguousarray(np.asarray(inputs["patch_features"], np.float32))
    W1 = np.asarray(inputs["W1"], np.float32)
    b1 = np.asarray(inputs["b1"], np.float32)
    W2 = np.asarray(inputs["W2"], np.float32).reshape(-1)
    b2 = float(np.asarray(inputs["b2"], np.float32).reshape(-1)[0])
    Wp = np.ascontiguousarray(np.asarray(inputs["Wp"], np.float32))
    bp = np.asarray(inputs["bp"], np.float32)
    gam = np.asarray(inputs["ln_gamma"], np.float32)
    bet = np.asarray(inputs["ln_beta"], np.float32)

    def _pack2(a):  # [256, X] -> [128, 2X]: row p = [a[p], a[p+128]]
        return np.ascontiguousarray(
            np.concatenate([a[:P], a[P:]], axis=1).astype(np.float16)
        )

    w1a = _pack2(W1[:D])
    w1b = _pack2(W1[D:])
    b1c = np.ascontiguousarray(b1.reshape(2, P).T)  # [P, 2]
    w2c = np.ascontiguousarray(W2.reshape(2, P).T.astype(np.float16))  # [P, 2]
    negid = np.ascontiguousarray(
        (np.eye(P) * (-30.0 * math.exp(b2))).astype(np.float16)
    )
    cid = np.ascontiguousarray((np.eye(P) * math.exp(-b2)).astype(np.float16))
    bpr = np.ascontiguousarray(np.broadcast_to(bp[None, :], (P, D)))
    use_gamma = not np.all(gam == 1.0)
    use_beta = not np.all(bet == 0.0)
    gamr = np.ascontiguousarray(np.broadcast_to(gam[None, :], (P, D)))

    common = {
        "w1a": w1a,
        "w1b": w1b,
        "b1c": b1c,
        "w2c": w2c,
        "negid": negid,
        "cid": cid,
        "wp": Wp.astype(np.float16),
        "bpr": bpr,
    }
    if use_gamma:
        common["gamr"] = gamr
    in_maps = []
    for b in range(B):
        m = dict(common)
        m["xb"] = np.ascontiguousarray(x[b])
        m["xt16"] = _pack2(x[b].T)
        m["x16"] = np.ascontiguousarray(x[b].astype(np.float16))
        if use_beta:
            m["xpb"] = np.ascontiguousarray(x[b] + bet[None, :])
        in_maps.append(m)
    return in_maps, b2, use_gamma, use_beta


def _run(inputs, trace=False, tmpdir=None):
    in_maps, b2, use_gamma, use_beta = _host_inputs(inputs)
    nc = _get_program(b2, use_gamma, use_beta)
    res = run_bass_kernel_spmd(
        nc, in_maps, list(range(B)), trace=trace, tmpdir=tmpdir
    )
    out = np.stack([res.results[b]["out"] for b in range(B)]).astype(np.float32)
    return out, res


def kernel(**inputs) -> np.ndarray:
    out, _ = _run(inputs)
    return out


def predicted_time_ns():
    """Cost-model timeline estimate of one core's NEFF execution (ns)."""
    from concourse.timeline_sim import TimelineSim

    assert _cache, "run the kernel first"
    nc = next(iter(_cache.values()))
    tl = TimelineSim(nc, trace=False)
    return int(tl.simulate())


# revision 16
# speedup vs baseline: 1.0077x; 1.0077x over previous
"""Trainium2 Bass kernel for nn_CrossPatchContextModule.

Math (per batch b):
    hi = x @ W1[:D];  hj = x @ W1[D:]
    scores[i,j] = W2 . relu(hi[i] + hj[j] + b1) + b2     (diag forced to 0)
    w = softmax(scores, axis=j)
    out = x + LN(w @ x @ Wp + bp) * gamma + beta

Sharding: data-parallel over batch. B=8 batches -> 8 NeuronCores, one
batch per core, all parameters replicated. No collectives.

Per-core algorithm (N=D=256, P=128 partitions):
  * hjW[e,j] = (W1[D:]^T x^T)[e,j] (fp16, streamed operand) and
    hibW[e,i] = (W1[:D]^T x^T)[e,i] + b1[e] (fp32, per-partition scalar
    source), each as two 128-row e-chunks.
  * Pairwise tiles are emitted per i (transposed relative to the score
    matrix): T_i[e, (c,j)] = relu(hjW[c][e,j] + hibW[c][e,i]) in ONE
    fused tensor_scalar (DVE / Pool) or activation-Relu (ACT) op per
    (i, c), [128, 256] fp16. i's are distributed over the three
    elementwise-capable engines in proportion to their per-op rates
    (DVE ~127ns, ACT ~398ns, Pool ~451ns per [128,256] op), two i's
    packed per SBUF tile to halve pool-recycle semaphore traffic.
  * scores^T[:, i] = sum_e W2_e T_i[e, :]: the PE contracts with T_i as
    the STATIONARY operand (lhsT [128e, 128j]) and the W2 e-chunk
    column [128, 1] as the moving operand, accumulating the two
    e-chunks into one PSUM column psum_sT[:, g*N+i] (j = p + 128g).
    The moving free size is 1, so the contraction occupies the PE for
    only ~1 cycle per matmul; weight loads ride the PE's spare
    capacity. Each column's accumulation opens with a one-hot matmul
    that plants -30 on the diagonal entry (j == i), so exp underflows
    fp16 to an exact 0 there and no masking ever touches the vector
    engines. PSUM holds scores TRANSPOSED: [j(part), i(free)].
  * softmax: b2 shifts every in-row score equally so it cancels in
    softmax EXCEPT at the zeroed diagonal. Exp runs straight on the
    raw PSUM scores; the diagonal's exp(-b2) re-enters as constant PE
    accumulations: a memset (exp(-b2)/128) tile into the row-sum S and
    exp(-b2)*I @ x^T into ctx. Row sums come from a ones-column PE
    matvec over ew^T; normalization by 1/S_i is folded into the proj
    output as (ctx_raw @ Wp) * r_i + bp in one scalar_tensor_tensor.
  * The i<128 half of the whole softmax/ctx/proj/LayerNorm/output
    pipeline is emitted MID-LOOP: its score columns complete halfway
    through the pairwise stage, so that half's tail (including its
    output DMA) hides entirely under the elementwise phase. Only the
    i>=128 half runs after the last pairwise tile.
  * ctx^T[d,i] = x^T-chunks (lhsT) @ ew^T, proj[i,e] = ctx^T (lhsT) @
    Wp; LayerNorm via bn_stats/bn_aggr, rstd = Exp(-0.5*Ln(var+eps)),
    y = (pb - mu)*rstd in one two-scalar tensor_scalar, residual add,
    fp16 output DMA (host widens to fp32).

T tiles, ew, the matmul operands and the output are fp16 (DVE
tensor_scalar gets the 4x 16-bit perf mode; the PE streams fp16 at
1 col/cycle). Scores accumulate in fp32 PSUM; LayerNorm statistics and
the residual math stay fp32.
"""

import math

import numpy as np
from contextlib import ExitStack

import concourse.bass as bass
import concourse.bacc as bacc
import concourse.tile as tile
from concourse import mybir
from concourse.bass_utils import run_bass_kernel_spmd

B, N, D = 8, 256, 256
P = 128
LN_EPS = 1e-5
F32 = mybir.dt.float32
F16 = mybir.dt.float16
AF = mybir.ActivationFunctionType
OP = mybir.AluOpType

# Tile schedule for the pairwise stage: tiles hold 1-2 i's, all ops of a
# tile on one engine. Engine shares match per-[128,256]-op rates (DVE
# 127ns, ACT 398ns, Pool 451ns): 160 i's DVE, 51 ACT, 45 Pool. Tiles are
# interleaved by earliest-virtual-finish so the three engine streams and
# the PE's in-order column consumption pace together.
def _tile_sched():
    per_op = {"v": 0.127, "a": 0.398, "p": 0.4505}
    tiles = (
        [("v", 2)] * 79 + [("a", 2)] * 25 + [("a", 1)] + [("p", 2)] * 23 + [("p", 1)]
    )
    by_eng = {k: [t for t in tiles if t[0] == k] for k in per_op}
    t_eng = {k: 0.0 for k in per_op}
    idx = {k: 0 for k in per_op}
    order = []
    next_i = 0
    for _ in range(len(tiles)):
        # pick engine whose next tile would finish earliest
        k = min(
            (k for k in per_op if idx[k] < len(by_eng[k])),
            key=lambda k: t_eng[k] + 2 * by_eng[k][idx[k]][1] * per_op[k],
        )
        n_i = by_eng[k][idx[k]][1]
        idx[k] += 1
        t_eng[k] += 2 * n_i * per_op[k]
        order.append((k, list(range(next_i, next_i + n_i))))
        next_i += n_i
    assert next_i == N
    return order


def _single_act_table(arch):
    """All activation funcs this kernel uses (Relu/Identity/Copy/Exp/Ln)
    live in set 6 (natural_log_exp_and_others). The stock greedy table
    placer picks sets 0/5 and ping-pongs 5 table loads (~2.7us each on
    HW); masking every other set forces one load of set 6. Canonical set
    indices are preserved (walrus maps id -> act.json by position)."""
    import concourse.hw_specs as hw_specs

    tabs = hw_specs.get_activation_tables(arch)
    keep = "natural_log_exp_and_others"
    need = {AF.Relu, AF.Identity, AF.Copy, AF.Exp, AF.Ln}
    if keep not in tabs or not need.issubset(tabs[keep]):
        return tabs  # fall back to the stock placement
    return {name: (funcs if name == keep else set()) for name, funcs in tabs.items()}


def _build_program(b2_val: float, use_gamma: bool, use_beta: bool):
    nc = bacc.Bacc("TRN2", target_bir_lowering=False, debug=False)

    xb_d = nc.dram_tensor("xb", [N, D], F32, kind="ExternalInput")
    xt16_d = nc.dram_tensor("xt16", [P, 2 * N], F16, kind="ExternalInput")
    x16_d = nc.dram_tensor("x16", [N, D], F16, kind="ExternalInput")
    w1a_d = nc.dram_tensor("w1a", [P, 2 * D], F16, kind="ExternalInput")
    w1b_d = nc.dram_tensor("w1b", [P, 2 * D], F16, kind="ExternalInput")
    b1c_d = nc.dram_tensor("b1c", [P, 2], F32, kind="ExternalInput")
    w2c_d = nc.dram_tensor("w2c", [P, 2], F16, kind="ExternalInput")
    # negid = (-30 * exp(b2)) * I, cid = exp(-b2) * I: negid[:, m] x cid
    # one-hot columns plant -30 on score diagonals; cid @ x^T re-adds the
    # diagonal softmax weight exp(-b2) into ctx.
    negid_d = nc.dram_tensor("negid", [P, P], F16, kind="ExternalInput")
    cid_d = nc.dram_tensor("cid", [P, P], F16, kind="ExternalInput")
    wp_d = nc.dram_tensor("wp", [D, D], F16, kind="ExternalInput")
    bpr_d = nc.dram_tensor("bpr", [P, D], F32, kind="ExternalInput")
    xpb_d = (
        nc.dram_tensor("xpb", [N, D], F32, kind="ExternalInput")
        if use_beta
        else None
    )
    gam_d = (
        nc.dram_tensor("gamr", [P, D], F32, kind="ExternalInput")
        if use_gamma
        else None
    )
    out_d = nc.dram_tensor("out", [N, D], F16, kind="ExternalOutput")

    with tile.TileContext(nc) as tc, ExitStack() as ctx:
        const = ctx.enter_context(tc.tile_pool(name="const", bufs=1))
        tpool = {
            "v": ctx.enter_context(tc.tile_pool(name="tv", bufs=12)),
            "a": ctx.enter_context(tc.tile_pool(name="ta", bufs=6)),
            "p": ctx.enter_context(tc.tile_pool(name="tp", bufs=6)),
        }
        ppre = ctx.enter_context(tc.tile_pool(name="ppre", bufs=2, space="PSUM"))
        pctx = ctx.enter_context(tc.tile_pool(name="pctx", bufs=1, space="PSUM"))
        pproj = ctx.enter_context(tc.tile_pool(name="pproj", bufs=1, space="PSUM"))
        pS = ctx.enter_context(tc.tile_pool(name="pS", bufs=1, space="PSUM"))
        pscore = ctx.enter_context(
            tc.tile_pool(name="pscore", bufs=1, space="PSUM")
        )

        # per-partition scalar constants for activation bias operands
        zero1 = const.tile([P, 1], F32)
        nc.vector.memset(zero1, 0.0)
        eps1 = const.tile([P, 1], F32)
        nc.vector.memset(eps1, LN_EPS)
        ones16 = const.tile([P, 1], F16)
        nc.vector.memset(ones16, 1.0)
        # cfill @ ones adds the diagonal's exp(-b2) into the softmax sums
        cfill = const.tile([P, P], F16)
        nc.vector.memset(cfill, math.exp(-b2_val) / P)
        # dummy activation with no data deps: forces the one ACT table load
        # (natural_log_exp set, ~1.3us) to run at t~0 instead of gating the
        # first real ACT op
        warm = const.tile([P, 1], F32)
        nc.scalar.activation(warm, zero1, AF.Relu, bias=zero1[:, 0:1])

        # ------- input DMAs needed before the main loop -------------------
        # x^T and the fp16 copy of x are host-side layout transforms of the
        # per-core shard (no FLOPs) - DMA them directly.
        # chunk pairs merged into single DMAs (DMA init latency dominates
        # these small transfers): tile [128, 2, 256], block c = rows
        # [128c, 128c+128) of the dram tensor
        # packed [P, 2X] layouts: one contiguous 1024B run per partition
        # per DMA descriptor (512B runs pay a 2x latency multiplier)
        xT_all = const.tile([P, 2 * N], F16)
        nc.sync.dma_start(xT_all, xt16_d[:])
        xT = [xT_all[:, c * N : (c + 1) * N] for c in range(2)]
        w1a_all = const.tile([P, 2 * D], F16)
        nc.scalar.dma_start(w1a_all, w1a_d[:])
        w1a = [w1a_all[:, c * D : (c + 1) * D] for c in range(2)]
        w1b_all = const.tile([P, 2 * D], F16)
        nc.gpsimd.dma_start(w1b_all, w1b_d[:])
        w1b = [w1b_all[:, c * D : (c + 1) * D] for c in range(2)]
        b1c = const.tile([P, 2], F32)
        nc.gpsimd.dma_start(b1c, b1c_d[:])
        # w2c/negid/cid only feed the PE score matmuls, which trail the
        # elementwise engines anyway - they can arrive late.
        w2c = const.tile([P, 2], F16)
        nc.gpsimd.dma_start(w2c, w2c_d[:])
        negid = const.tile([P, P], F16)
        nc.sync.dma_start(negid, negid_d[:])
        cid = const.tile([P, P], F16)
        nc.sync.dma_start(cid, cid_d[:])
        # ---------------- hjW (fp16 stream), hibW (fp32 scalars) ----------
        # hjW[e,j] = sum_d W1b[d,e] x[j,d] ; hibW[e,i] = sum_d W1a[d,e]
        # x[i,d] + b1[e]. ec=0 chunks first so the c=0 pairwise ops can
        # start while the ec=1 matmuls still run.
        hjW = [const.tile([P, N], F16, tag=f"hjW{c}", name=f"hjW{c}") for c in range(2)]
        hibW = [const.tile([P, N], F32, tag=f"hibW{c}", name=f"hibW{c}") for c in range(2)]
        for ec in range(2):
            ph = ppre.tile([P, N], F32, tag="mm")
            for dc in range(2):
                nc.tensor.matmul(
                    ph,
                    w1b[dc][:, ec * P : (ec + 1) * P],
                    xT[dc],
                    start=(dc == 0),
                    stop=(dc == 1),
                )
            nc.scalar.activation(hjW[ec], ph, AF.Identity, bias=zero1[:, 0:1])
            ph2 = ppre.tile([P, N], F32, tag="mm")
            for dc in range(2):
                nc.tensor.matmul(
                    ph2,
                    w1a[dc][:, ec * P : (ec + 1) * P],
                    xT[dc],
                    start=(dc == 0),
                    stop=(dc == 1),
                )
            nc.vector.tensor_scalar(
                out=hibW[ec], in0=ph2, scalar1=b1c[:, ec : ec + 1],
                scalar2=None, op0=OP.add,
            )

        # ---------------- pairwise scores (transposed) --------------------
        # psum_sT[p, g*256+i] = scores[i, j=p+128g] (diag = -30)
        # tile layout per i: [c0: j 0..256 | c1: j 0..256], each 256 wide
        psum_sT = pscore.tile([P, 2 * N], F32)
        engs = {"v": nc.vector, "a": nc.scalar, "p": nc.gpsimd}

        def emit_op(ek, tt, k, i, c):
            dst = tt[:, k, c, :]
            if ek == "a":
                nc.scalar.activation(
                    dst, hjW[c], AF.Relu, bias=hibW[c][:, i : i + 1]
                )
            else:
                engs[ek].tensor_scalar(
                    out=dst,
                    in0=hjW[c],
                    scalar1=hibW[c][:, i : i + 1],
                    scalar2=0.0,
                    op0=OP.add,
                    op1=OP.max,
                )

        def emit_mms(tt, k, i):
            gd = i // P
            for g in range(2):
                col = psum_sT[:, g * N + i : g * N + i + 1]
                if g == gd:
                    nc.tensor.matmul(
                        col, cid, negid[:, i % P : i % P + 1],
                        start=True, stop=False, skip_group_check=True,
                    )
                for c in range(2):
                    nc.tensor.matmul(
                        col,
                        tt[:, k, c, g * P : g * P + P],
                        w2c[:, c : c + 1],
                        start=(c == 0 and g != gd),
                        stop=(c == 1),
                        skip_group_check=True,
                    )

        # ---- tail stages, per i-half (ic=0 emitted mid-loop) -------------
        ew = const.tile([P, 2, N], F16)
        S_ps = pS.tile([P, 2], F32)
        recip = const.tile([P, 2], F32)
        ctxT = [const.tile([P, N], F16, tag=f"ctxT{c}", name=f"ctxT{c}") for c in range(2)]
        pcs = [None, None]
        tail_state = {}

        def emit_tail(ic):
            isl = slice(ic * P, (ic + 1) * P)
            ps2 = psum_sT[:].rearrange("p (g n) -> p g n", g=2)
            nc.scalar.activation(
                ew[:, :, isl], ps2[:, :, isl], AF.Exp, bias=zero1[:, 0:1]
            )
            # softmax denominators: S[i] = sum_j ew^T[j, i] + exp(-b2)
            for g in range(2):
                nc.tensor.matmul(
                    S_ps[:, ic : ic + 1],
                    ew[:, g, isl],
                    ones16[:, 0:1],
                    start=(g == 0),
                    stop=False,
                    skip_group_check=True,
                )
            nc.tensor.matmul(
                S_ps[:, ic : ic + 1], cfill, ones16[:, 0:1],
                start=False, stop=True, skip_group_check=True,
            )
            nc.vector.reciprocal(recip[:, ic : ic + 1], S_ps[:, ic : ic + 1])
            # ctx^T[d, i] = sum_j x[j, d] ew^T[j, i] + exp(-b2) x^T[d, i]
            for dc in range(2):
                if pcs[dc] is None:
                    pcs[dc] = pctx.tile([P, N], F32, tag=f"pc{dc}", name=f"pc{dc}")
                pc = pcs[dc]
                for g in range(2):
                    nc.tensor.matmul(
                        pc[:, isl],
                        x16[g][:, dc * P : (dc + 1) * P],
                        ew[:, g, isl],
                        start=(g == 0),
                        stop=False,
                        skip_group_check=True,
                    )
                nc.tensor.matmul(
                    pc[:, isl], cid, xT[dc][:, isl],
                    start=False, stop=True, skip_group_check=True,
                )
                if dc == 0:
                    nc.vector.tensor_copy(ctxT[dc][:, isl], pc[:, isl])
                else:
                    nc.scalar.copy(ctxT[dc][:, isl], pc[:, isl])
            # proj (raw), then pb = proj*r_i + bp, LayerNorm, residual
            pp = pproj.tile([P, N], F32, tag="pp", name=f"pp{ic}")
            for dc in range(2):
                nc.tensor.matmul(
                    pp,
                    ctxT[dc][:, isl],
                    wp16[dc],
                    start=(dc == 0),
                    stop=(dc == 1),
                )
            pb = const.tile([P, D], F32, tag=f"pb{ic}", name=f"pb{ic}")
            nc.vector.scalar_tensor_tensor(
                out=pb, in0=pp, scalar=recip[:, ic : ic + 1], in1=bpr,
                op0=OP.mult, op1=OP.add,
            )
            st = const.tile([P, 6], F32, tag=f"st{ic}", name=f"st{ic}")
            nc.vector.bn_stats(st, pb)
            mv = const.tile([P, 2], F32, tag=f"mv{ic}", name=f"mv{ic}")
            nc.vector.bn_aggr(mv, st)
            lnv = const.tile([P, 1], F32, tag=f"lnv{ic}", name=f"lnv{ic}")
            nc.scalar.activation(lnv, mv[:, 1:2], AF.Ln, bias=eps1[:, 0:1])
            rstd = const.tile([P, 1], F32, tag=f"rstd{ic}", name=f"rstd{ic}")
            nc.scalar.activation(rstd, lnv, AF.Exp, bias=zero1[:, 0:1], scale=-0.5)
            # y = (pb - mu) * rstd in one two-scalar op
            tt2 = const.tile([P, D], F32, tag=f"tt{ic}", name=f"tt{ic}")
            tt_eng = nc.vector
            tt_eng.tensor_scalar(
                out=tt2,
                in0=pb,
                scalar1=mv[:, 0:1],
                scalar2=rstd[:, 0:1],
                op0=OP.subtract,
                op1=OP.mult,
            )
            if use_gamma:
                tg = const.tile([P, D], F32, tag=f"tg{ic}", name=f"tg{ic}")
                nc.vector.tensor_tensor(out=tg, in0=tt2, in1=gam, op=OP.mult)
                tt2 = tg
            ot = const.tile([P, D], F16, tag=f"ot{ic}", name=f"ot{ic}")
            ot_eng = nc.vector
            ot_eng.tensor_tensor(out=ot, in0=tt2, in1=xpb[ic], op=OP.add)
            nc.sync.dma_start(out_d[ic * P : (ic + 1) * P, :], ot)

        sched = _tile_sched()
        # find the tile index after which all i<128 columns are complete,
        # plus a safety margin so the PE has surely caught up
        done_i = 0
        tail0_at = None
        for m, (ek, ii) in enumerate(sched):
            done_i = max(done_i, max(ii) + 1)
            if done_i >= P and tail0_at is None:
                tail0_at = m + 6
        # stagger the first DVE tiles: their c=0 ops run while the ec=1
        # projections still compute, so DVE never stalls on hjW[1]
        pending = []  # staggered DVE tiles whose c=1 ops are deferred
        n_stag = 0
        tail_dmas_emitted = False
        for m, (ek, ii) in enumerate(sched):
            if m == 2 and not tail_dmas_emitted:
                # DMAs needed by the tail; emitted once the front DMAs of
                # each queue are already in flight so these queue behind
                tail_dmas_emitted = True
                x = [
                    const.tile([P, D], F32, tag=f"x{c}", name=f"x{c}")
                    for c in range(2)
                ]
                nc.sync.dma_start(x[0], xb_d[0:P, :])
                nc.sync.dma_start(x[1], xb_d[P : 2 * P, :])
                x16_all = const.tile([P, 2, D], F16)
                nc.sync.dma_start(
                    x16_all, x16_d[:].rearrange("(c p) n -> p c n", p=P)
                )
                x16 = [x16_all[:, c, :] for c in range(2)]
                wp16_all = const.tile([P, 2, D], F16)
                nc.sync.dma_start(
                    wp16_all, wp_d[:].rearrange("(c p) n -> p c n", p=P)
                )
                wp16 = [wp16_all[:, c, :] for c in range(2)]
                bpr = const.tile([P, D], F32)
                nc.sync.dma_start(bpr, bpr_d[:])
                if use_beta:
                    xpb = [
                        const.tile([P, D], F32, tag=f"xpb{c}", name=f"xpb{c}")
                        for c in range(2)
                    ]
                    for c in range(2):
                        nc.sync.dma_start(xpb[c], xpb_d[c * P : (c + 1) * P, :])
                else:
                    xpb = x
                if use_gamma:
                    gam = const.tile([P, D], F32)
                    nc.sync.dma_start(gam, gam_d[:])
            tt = tpool[ek].tile([P, len(ii), 2, N], F16, tag=f"T{ek}{len(ii)}")
            if ek == "v" and n_stag < 6:
                for k, i in enumerate(ii):
                    emit_op(ek, tt, k, i, 0)
                pending.append((ek, tt, list(enumerate(ii))))
                n_stag += 1
                continue
            for k, i in enumerate(ii):
                emit_op(ek, tt, k, i, 0)
                emit_op(ek, tt, k, i, 1)
            for k, i in enumerate(ii):
                emit_mms(tt, k, i)
            if pending and n_stag == 6:
                for pek, ptt, pki in pending:
                    for k, i in pki:
                        emit_op(pek, ptt, k, i, 1)
                    for k, i in pki:
                        emit_mms(ptt, k, i)
                pending = []
                n_stag = 7
            if m == tail0_at:
                emit_tail(0)
        emit_tail(1)

    import concourse.bacc as _bacc_mod

    orig = _bacc_mod.get_activation_tables
    _bacc_mod.get_activation_tables = _single_act_table
    try:
        nc.compile()
    finally:
        _bacc_mod.get_activation_tables = orig
    return nc


_cache = {}


def _get_program(b2_val: float, use_gamma: bool, use_beta: bool):
    key = (b2_val, use_gamma, use_beta)
    if key not in _cache:
        _cache[key] = _build_program(b2_val, use_gamma, use_beta)
    return _cache[key]


def _host_inputs(inputs):
    x = np.ascontiguousarray(np.asarray(inputs["patch_features"], np.float32))
    W1 = np.asarray(inputs["W1"], np.float32)
    b1 = np.asarray(inputs["b1"], np.float32)
    W2 = np.asarray(inputs["W2"], np.float32).reshape(-1)
    b2 = float(np.asarray(inputs["b2"], np.float32).reshape(-1)[0])
    Wp = np.ascontiguousarray(np.asarray(inputs["Wp"], np.float32))
    bp = np.asarray(inputs["bp"], np.float32)
    gam = np.asarray(inputs["ln_gamma"], np.float32)
    bet = np.asarray(inputs["ln_beta"], np.float32)

    def _pack2(a):  # [256, X] -> [128, 2X]: row p = [a[p], a[p+128]]
        return np.ascontiguousarray(
            np.concatenate([a[:P], a[P:]], axis=1).astype(np.float16)
        )

    w1a = _pack2(W1[:D])
    w1b = _pack2(W1[D:])
    b1c = np.ascontiguousarray(b1.reshape(2, P).T)  # [P, 2]
    w2c = np.ascontiguousarray(W2.reshape(2, P).T.astype(np.float16))  # [P, 2]
    negid = np.ascontiguousarray(
        (np.eye(P) * (-30.0 * math.exp(b2))).astype(np.float16)
    )
    cid = np.ascontiguousarray((np.eye(P) * math.exp(-b2)).astype(np.float16))
    bpr = np.ascontiguousarray(np.broadcast_to(bp[None, :], (P, D)))
    use_gamma = not np.all(gam == 1.0)
    use_beta = not np.all(bet == 0.0)
    gamr = np.ascontiguousarray(np.broadcast_to(gam[None, :], (P, D)))

    common = {
        "w1a": w1a,
        "w1b": w1b,
        "b1c": b1c,
        "w2c": w2c,
        "negid": negid,
        "cid": cid,
        "wp": Wp.astype(np.float16),
        "bpr": bpr,
    }
    if use_gamma:
        common["gamr"] = gamr
    in_maps = []
    for b in range(B):
        m = dict(common)
        m["xb"] = np.ascontiguousarray(x[b])
        m["xt16"] = _pack2(x[b].T)
        m["x16"] = np.ascontiguousarray(x[b].astype(np.float16))
        if use_beta:
            m["xpb"] = np.ascontiguousarray(x[b] + bet[None, :])
        in_maps.append(m)
    return in_maps, b2, use_gamma, use_beta


def _run(inputs, trace=False, tmpdir=None):
    in_maps, b2, use_gamma, use_beta = _host_inputs(inputs)
    nc = _get_program(b2, use_gamma, use_beta)
    res = run_bass_kernel_spmd(
        nc, in_maps, list(range(B)), trace=trace, tmpdir=tmpdir
    )
    out = np.stack([res.results[b]["out"] for b in range(B)]).astype(np.float32)
    return out, res


def kernel(**inputs) -> np.ndarray:
    out, _ = _run(inputs)
    return out


def predicted_time_ns():
    """Cost-model timeline estimate of one core's NEFF execution (ns)."""
    from concourse.timeline_sim import TimelineSim

    assert _cache, "run the kernel first"
    nc = next(iter(_cache.values()))
    tl = TimelineSim(nc, trace=False)
    return int(tl.simulate())
